# revision 9
# baseline (speedup 1.0000x reference)
"""Trainium2 Bass kernel for AlignedQuestionEmbeddingLayer.

Computation (per batch element):
    C = relu(Xc @ W.T + b)            # [4096, 128]
    Q = relu(Xq @ W.T + b)            # [512, 128]
    S = C @ Q.T  (+ mask)             # [4096, 512]
    A = softmax(S, axis=-1)
    out = A @ Q                       # [4096, 128]

Sharding: data-parallel over batch, one batch element per NeuronCore (8 cores).

Device-side design notes:
  - The dense layer contracts over E=300, so both matmul operands need E on
    the partition axis. Inputs are pre-transposed on host ([E, M] layout) so
    no on-device transposes of the big tensor are needed.
  - Scores are computed transposed ([q partitions, c free]) so the question
    mask folds into the exp() bias operand (per-partition) for free, and the
    final matmul (lhsT=expS_T chunk, rhs=[Q|ones]) produces both the output
    rows and the softmax denominators in a single PSUM accumulation.
  - Softmax skips max-subtraction: scores are bounded (~|s|<35) so exp() is
    safe in fp32; this matches the reference up to fp rounding.
  - fp32 matmuls are self-loading on TRN2 and only tolerate one sync-wait
    each, so: all setup constants ship in a single blob DMA, Xc loads use
    two DMAs per super-tile, and a tiny absorber matmul per super-tile
    carries the PSUM-slot-reuse wait on the DVE normalizer.
"""

import sys

import numpy as np

sys.path.insert(0, "/opt/trn_rl_repo")

B, CTX, QST, E, H = 8, 4096, 512, 300, 128
N_CORES = 8
ST = 512            # context rows per super-tile
N_ST = CTX // ST    # 8 super-tiles
EP = 384            # E padded to 3 chunks of 128 (blob layout only)

# blob free-dim layout (one [128, BLOB_F] f32 DMA carries all constants)
OFF_W = 0                 # wT chunks:  [:, OFF_W + k*128 : +128], k=0..2
OFF_Q = 384               # qsT chunks: [:, OFF_Q + k*512 : +512], k=0..2
OFF_ID = OFF_Q + 3 * 512  # identity 128x128
OFF_B = OFF_ID + 128      # bias column [128, 1]
OFF_MB = OFF_B + 1        # mask bias columns [128, 4] (j-th chunk of 512)
BLOB_F = OFF_MB + 4

_COMPILED = {}


def _build_kernel(n_st=N_ST):
    import concourse.bass as bass
    import concourse.tile as tile
    from concourse import bacc, mybir

    f32 = mybir.dt.float32
    AF = mybir.ActivationFunctionType

    nc = bacc.Bacc(
        "TRN2", target_bir_lowering=False, debug=False, num_devices=N_CORES
    )

    xcT = nc.declare_dram_parameter("xcT", [E, CTX], f32, isOutput=False)
    blob_d = nc.declare_dram_parameter("blob", [128, BLOB_F], f32, isOutput=False)
    out_d = nc.declare_dram_parameter("out", [CTX, H], f32, isOutput=True)

    with tile.TileContext(nc) as tc:
        with (
            tc.tile_pool(name="const", bufs=1) as const_pool,
            tc.tile_pool(name="xin", bufs=3) as xin_pool,
            tc.tile_pool(name="ct", bufs=2) as ct_pool,
            tc.tile_pool(name="exps", bufs=2) as exps_pool,
            tc.tile_pool(name="outs", bufs=4) as outs_pool,
            tc.tile_pool(name="pct", bufs=1, space=bass.MemorySpace.PSUM) as pct_pool,
            tc.tile_pool(name="pst", bufs=3, space=bass.MemorySpace.PSUM) as pst_pool,
            tc.tile_pool(name="po", bufs=4, space=bass.MemorySpace.PSUM) as po_pool,
        ):
            # ---- one-time setup -------------------------------------------
            setup = const_pool.tile([128, BLOB_F], f32, tag="setup")
            nc.sync.dma_start(setup[:], blob_d[:])

            def w_chunk(k):  # [ke, 128] lhsT for dense (ke rows valid)
                ke = 128 if k < 2 else E - 256
                return setup[0:ke, OFF_W + k * 128 : OFF_W + (k + 1) * 128]

            def q_chunk(k):  # [ke, 512] rhs for question dense
                ke = 128 if k < 2 else E - 256
                return setup[0:ke, OFF_Q + k * 512 : OFF_Q + (k + 1) * 512]

            ident = setup[:, OFF_ID : OFF_ID + 128]
            bias_col = setup[:, OFF_B : OFF_B + 1]

            # question dense: QT = relu(W @ XqT + b) -> [H, QST] (h on parts)
            psum_q = pct_pool.tile([H, QST], f32, tag="pct")
            for k in range(3):
                nc.tensor.matmul(
                    psum_q[:], w_chunk(k), q_chunk(k), start=(k == 0), stop=(k == 2)
                )
            qt_sb = const_pool.tile([H, QST], f32, tag="qt")
            nc.scalar.activation(qt_sb[:], psum_q[:], AF.Relu, bias=bias_col)

            # Qa[j] = [Q_chunk | ones] -> [128 q, H+1]; Q = QT.T via PE transpose
            qa_sb = []
            for j in range(4):
                pt = po_pool.tile([128, H + 1], f32, tag="po")
                nc.tensor.transpose(
                    pt[:, 0:128], qt_sb[:, j * 128 : (j + 1) * 128], ident
                )
                qa = const_pool.tile([128, H + 1], f32, tag=f"qa{j}")
                nc.scalar.activation(qa[:, 0:H], pt[:, 0:128], AF.Copy)
                # ones column, written on ACT: Copy(in*0 + 1) = 1
                nc.scalar.activation(
                    qa[:, H : H + 1], bias_col, AF.Copy, bias=1.0, scale=0.0
                )
                qa_sb.append(qa)

            # ---- main loop over context super-tiles -----------------------

            for st in range(n_st):
                c0 = st * ST
                # Xc chunk loads: one DMA for e-rows 0:256, one for 256:300
                xa = xin_pool.tile([128, 2, ST], f32, tag="xa")
                src = xcT[0:256, c0 : c0 + ST].rearrange("(k p) c -> p k c", p=128)
                nc.sync.dma_start(xa[:], src)
                xb = xin_pool.tile([E - 256, ST], f32, tag="xb")
                nc.sync.dma_start(xb[:], xcT[256:E, c0 : c0 + ST])

                # context dense: CT = relu(W @ XcT + b) -> [H, ST]
                psum_ct = pct_pool.tile([H, ST], f32, tag="pct")
                for k in range(3):
                    rhs = xa[:, k, :] if k < 2 else xb[:]
                    nc.tensor.matmul(
                        psum_ct[:], w_chunk(k), rhs, start=(k == 0), stop=(k == 2)
                    )
                ct_sb = ct_pool.tile([H, ST], f32, tag="ct")
                nc.scalar.activation(ct_sb[:], psum_ct[:], AF.Relu, bias=bias_col)

                # transposed scores + exp: expST[j] = exp(QT_j.T @ CT + mb_j)
                exp_sb = []
                for j in range(4):
                    ps = pst_pool.tile([128, ST], f32, tag="pst")
                    nc.tensor.matmul(
                        ps[:], qt_sb[:, j * 128 : (j + 1) * 128], ct_sb[:],
                        start=True, stop=True,
                    )
                    es = exps_pool.tile([128, ST], f32, tag=f"e{j}")
                    nc.scalar.activation(
                        es[:], ps[:], AF.Exp,
                        bias=setup[:, OFF_MB + j : OFF_MB + j + 1],
                    )
                    exp_sb.append(es)

                # final: out[c,:H] and softmax denominator in one accumulation.
                # The po tile is read ONLY by the DVE evict, so the slot-reuse
                # wait on the j==0 matmul is a single DVE wait (its exp[0]
                # dependency is already covered by the scores j==3 matmul's
                # pst-slot wait on exp[0]).
                for ci in range(ST // 128):
                    po = po_pool.tile([128, H + 1], f32, tag="po")
                    for j in range(4):
                        nc.tensor.matmul(
                            po[:],
                            exp_sb[j][:, ci * 128 : (ci + 1) * 128],
                            qa_sb[j][:],
                            start=(j == 0), stop=(j == 3),
                        )
                    o_raw = outs_pool.tile([128, H + 1], f32, tag="oraw")
                    nc.vector.tensor_copy(o_raw[:], po[:])
                    recip = outs_pool.tile([128, 1], f32, tag="recip")
                    nc.vector.reciprocal(recip[:], o_raw[:, H : H + 1])
                    o_sb = outs_pool.tile([128, H], f32, tag="osb")
                    nc.vector.tensor_scalar_mul(o_sb[:], o_raw[:, 0:H], recip[:])
                    nc.sync.dma_start(
                        out_d[c0 + ci * 128 : c0 + (ci + 1) * 128, :], o_sb[:]
                    )

    return nc


def _get_nc():
    if "nc" not in _COMPILED:
        nc = _build_kernel()
        nc.compile()
        nc.finalize()
        _COMPILED["nc"] = nc
    return _COMPILED["nc"]


def make_blob(W, b, question_sequence_i, question_mask_i):
    """Pack per-core constants into the single [128, BLOB_F] f32 setup blob."""
    blob = np.zeros((128, BLOB_F), np.float32)
    wTp = np.zeros((EP, H), np.float32)
    wTp[:E] = W.astype(np.float32).T
    for k in range(3):
        blob[:, OFF_W + k * 128 : OFF_W + (k + 1) * 128] = wTp[k * 128 : (k + 1) * 128]
    qTp = np.zeros((EP, QST), np.float32)
    qTp[:E] = question_sequence_i.astype(np.float32).T
    for k in range(3):
        blob[:, OFF_Q + k * 512 : OFF_Q + (k + 1) * 512] = qTp[k * 128 : (k + 1) * 128]
    blob[:, OFF_ID : OFF_ID + 128] = np.eye(128, dtype=np.float32)
    blob[:, OFF_B] = b.astype(np.float32)
    mb = np.where(question_mask_i == 0, np.float32(-1e30), np.float32(0.0))
    blob[:, OFF_MB : OFF_MB + 4] = mb.reshape(4, 128).T
    return blob


def make_in_maps(context_sequence, question_sequence, question_mask, W, b):
    in_maps = []
    for i in range(N_CORES):
        xcT = np.ascontiguousarray(context_sequence[i].astype(np.float32).T)
        blob = make_blob(W, b, question_sequence[i], question_mask[i])
        in_maps.append({"xcT": xcT, "blob": blob})
    return in_maps


def kernel(context_sequence, question_sequence, question_mask, W, b):
    from concourse.bass_utils import run_bass_kernel_spmd

    nc = _get_nc()
    in_maps = make_in_maps(
        context_sequence, question_sequence, question_mask, W, b)
    res = run_bass_kernel_spmd(nc, in_maps, core_ids=list(range(N_CORES)))
    out = np.stack([res.results[i]["out"] for i in range(N_CORES)], axis=0)
    return out.astype(np.float32)


# revision 10
# speedup vs baseline: 2.3928x; 2.3928x over previous
"""Trainium2 Bass kernel for AlignedQuestionEmbeddingLayer.

Computation (per batch element):
    C = relu(Xc @ W.T + b)            # [4096, 128]
    Q = relu(Xq @ W.T + b)            # [512, 128]
    S = C @ Q.T  (+ mask)             # [4096, 512]
    A = softmax(S, axis=-1)
    out = A @ Q                       # [4096, 128]

Sharding: data-parallel over batch, one batch element per NeuronCore (8 cores).

Device-side design notes:
  - The dense layer contracts over E=300, so both matmul operands need E on
    the partition axis. Inputs are pre-transposed on host ([E, M] layout) so
    no on-device transposes of the big tensor are needed.
  - fp32 matmuls run at 4 cycles/row on TRN2 vs 1 for 16-bit dtypes, so the
    dense runs on fp16 inputs (host-cast, also halves input DMA traffic),
    scores on fp16 (relu writes fp16), and the final matmul on bf16
    (exp values overflow fp16's range; bf16 keeps fp32 range). All PSUM
    accumulation is fp32.
  - Scores are computed transposed ([q partitions, c free]) so the question
    mask folds into the exp() bias operand (per-partition) for free, and the
    final matmul (lhsT=expS_T chunk, rhs=[Q|ones]) produces both the output
    rows and the softmax denominators in a single PSUM accumulation.
  - Softmax skips max-subtraction: scores are bounded (~|s|<40) so exp() is
    safe in fp32/bf16; this matches the reference up to fp rounding.
  - relu(+bias) runs on DVE (tensor_scalar add+max) to keep ACT free for
    the 32 exp() activations, which are the serial ACT cost.
"""

import sys

import numpy as np

sys.path.insert(0, "/opt/trn_rl_repo")

B, CTX, QST, E, H = 8, 4096, 512, 300, 128
N_CORES = 8
ST = 512            # context rows per super-tile
N_ST = CTX // ST    # 8 super-tiles

# fp16 blob free-dim layout: wT chunks, qsT chunks, identity
OFF_W = 0                 # [:, OFF_W + k*128 : +128], k=0..2
OFF_Q = 384               # [:, OFF_Q + k*512 : +512], k=0..2
OFF_ID = OFF_Q + 3 * 512  # identity 128x128
BLOB16_F = OFF_ID + 128
# fp32 blob: bias column + 4 maskbias columns
OFF_B = 0
OFF_MB = 1
BLOB32_F = 5

_COMPILED = {}


def _build_kernel(n_st=N_ST):
    import concourse.bass as bass
    import concourse.tile as tile
    from concourse import bacc, mybir

    f32 = mybir.dt.float32
    f16 = mybir.dt.float16
    bf16 = mybir.dt.bfloat16
    AF = mybir.ActivationFunctionType
    ADD = mybir.AluOpType.add
    MAX = mybir.AluOpType.max

    nc = bacc.Bacc(
        "TRN2", target_bir_lowering=False, debug=False, num_devices=N_CORES
    )

    xcT = nc.declare_dram_parameter("xcT", [E, CTX], f16, isOutput=False)
    b16_d = nc.declare_dram_parameter("b16", [128, BLOB16_F], f16, isOutput=False)
    b32_d = nc.declare_dram_parameter("b32", [128, BLOB32_F], f32, isOutput=False)
    out_d = nc.declare_dram_parameter("out", [CTX, H], f32, isOutput=True)

    with tile.TileContext(nc) as tc:
        with (
            tc.tile_pool(name="const", bufs=1) as const_pool,
            tc.tile_pool(name="xin", bufs=3) as xin_pool,
            tc.tile_pool(name="ct", bufs=2) as ct_pool,
            tc.tile_pool(name="exps", bufs=2) as exps_pool,
            tc.tile_pool(name="outs", bufs=4) as outs_pool,
            tc.tile_pool(name="pct", bufs=1, space=bass.MemorySpace.PSUM) as pct_pool,
            tc.tile_pool(name="pst", bufs=3, space=bass.MemorySpace.PSUM) as pst_pool,
            tc.tile_pool(name="po", bufs=4, space=bass.MemorySpace.PSUM) as po_pool,
        ):
            # ---- one-time setup -------------------------------------------
            setup16 = const_pool.tile([128, BLOB16_F], f16, tag="setup16")
            nc.sync.dma_start(setup16[:], b16_d[:])
            setup32 = const_pool.tile([128, BLOB32_F], f32, tag="setup32")
            nc.sync.dma_start(setup32[:], b32_d[:])

            def w_chunk(k):  # [ke, 128] fp16 lhsT for dense
                ke = 128 if k < 2 else E - 256
                return setup16[0:ke, OFF_W + k * 128 : OFF_W + (k + 1) * 128]

            def q_chunk(k):  # [ke, 512] fp16 rhs for question dense
                ke = 128 if k < 2 else E - 256
                return setup16[0:ke, OFF_Q + k * 512 : OFF_Q + (k + 1) * 512]

            ident = setup16[:, OFF_ID : OFF_ID + 128]
            bias_col = setup32[:, OFF_B : OFF_B + 1]

            # question dense: QT = relu(W @ XqT + b) -> [H, QST] fp16
            psum_q = pct_pool.tile([H, QST], f32, tag="pct")
            for k in range(3):
                nc.tensor.matmul(
                    psum_q[:], w_chunk(k), q_chunk(k), start=(k == 0), stop=(k == 2)
                )
            qt_sb = const_pool.tile([H, QST], f16, tag="qt")
            nc.vector.tensor_scalar(qt_sb[:], psum_q[:], bias_col, 0.0, ADD, MAX)

            # Qa[j] = [Q_chunk | ones] -> [128 q, H+1] bf16 (PE transpose of QT)
            qa_sb = []
            for j in range(4):
                pt = po_pool.tile([128, H + 1], f16, tag="po")
                nc.tensor.transpose(
                    pt[:, 0:128], qt_sb[:, j * 128 : (j + 1) * 128], ident
                )
                qa = const_pool.tile([128, H + 1], bf16, tag=f"qa{j}")
                nc.scalar.activation(qa[:, 0:H], pt[:, 0:128], AF.Copy)
                # ones column, written on ACT: Copy(in*0 + 1) = 1
                nc.scalar.activation(
                    qa[:, H : H + 1], bias_col, AF.Copy, bias=1.0, scale=0.0
                )
                qa_sb.append(qa)

            # ---- main loop over context super-tiles -----------------------
            for st in range(n_st):
                c0 = st * ST
                # Xc chunk loads: one DMA for e-rows 0:256, one for 256:300
                xa = xin_pool.tile([128, 2, ST], f16, tag="xa")
                src = xcT[0:256, c0 : c0 + ST].rearrange("(k p) c -> p k c", p=128)
                nc.sync.dma_start(xa[:], src)
                xb = xin_pool.tile([E - 256, ST], f16, tag="xb")
                nc.sync.dma_start(xb[:], xcT[256:E, c0 : c0 + ST])

                # context dense: CT = relu(W @ XcT + b) -> [H, ST] fp16 (DVE)
                psum_ct = pct_pool.tile([H, ST], f32, tag="pct")
                for k in range(3):
                    rhs = xa[:, k, :] if k < 2 else xb[:]
                    nc.tensor.matmul(
                        psum_ct[:], w_chunk(k), rhs, start=(k == 0), stop=(k == 2)
                    )
                ct_sb = ct_pool.tile([H, ST], f16, tag="ct")
                nc.vector.tensor_scalar(ct_sb[:], psum_ct[:], bias_col, 0.0, ADD, MAX)

                # transposed scores + exp -> bf16: expST[j] = exp(QTj.T@CT + mb_j)
                exp_sb = []
                for j in range(4):
                    ps = pst_pool.tile([128, ST], f32, tag="pst")
                    nc.tensor.matmul(
                        ps[:], qt_sb[:, j * 128 : (j + 1) * 128], ct_sb[:],
                        start=True, stop=True,
                    )
                    es = exps_pool.tile([128, ST], bf16, tag=f"e{j}")
                    nc.scalar.activation(
                        es[:], ps[:], AF.Exp,
                        bias=setup32[:, OFF_MB + j : OFF_MB + j + 1],
                    )
                    exp_sb.append(es)

                # final: out[c,:H] and softmax denominator in one accumulation
                for ci in range(ST // 128):
                    po = po_pool.tile([128, H + 1], f32, tag="po")
                    for j in range(4):
                        nc.tensor.matmul(
                            po[:],
                            exp_sb[j][:, ci * 128 : (ci + 1) * 128],
                            qa_sb[j][:],
                            start=(j == 0), stop=(j == 3),
                        )
                    recip = outs_pool.tile([128, 1], f32, tag="recip")
                    nc.vector.reciprocal(recip[:], po[:, H : H + 1])
                    o_sb = outs_pool.tile([128, H], f32, tag="osb")
                    nc.vector.tensor_scalar_mul(o_sb[:], po[:, 0:H], recip[:])
                    nc.sync.dma_start(
                        out_d[c0 + ci * 128 : c0 + (ci + 1) * 128, :], o_sb[:]
                    )

    return nc


def _get_nc():
    if "nc" not in _COMPILED:
        nc = _build_kernel()
        nc.compile()
        nc.finalize()
        _COMPILED["nc"] = nc
    return _COMPILED["nc"]


def make_blobs(W, b, question_sequence_i, question_mask_i):
    """Pack per-core constants into the fp16 and fp32 setup blobs."""
    b16 = np.zeros((128, BLOB16_F), np.float16)
    wTp = np.zeros((384, H), np.float16)
    wTp[:E] = W.astype(np.float16).T
    for k in range(3):
        b16[:, OFF_W + k * 128 : OFF_W + (k + 1) * 128] = wTp[k * 128 : (k + 1) * 128]
    qTp = np.zeros((384, QST), np.float16)
    qTp[:E] = question_sequence_i.astype(np.float16).T
    for k in range(3):
        b16[:, OFF_Q + k * 512 : OFF_Q + (k + 1) * 512] = qTp[k * 128 : (k + 1) * 128]
    b16[:, OFF_ID : OFF_ID + 128] = np.eye(128, dtype=np.float16)

    b32 = np.zeros((128, BLOB32_F), np.float32)
    b32[:, OFF_B] = b.astype(np.float32)
    mb = np.where(question_mask_i == 0, np.float32(-1e30), np.float32(0.0))
    b32[:, OFF_MB : OFF_MB + 4] = mb.reshape(4, 128).T
    return b16, b32


def make_in_maps(context_sequence, question_sequence, question_mask, W, b):
    in_maps = []
    for i in range(N_CORES):
        xcT = np.ascontiguousarray(context_sequence[i].T.astype(np.float16))
        b16, b32 = make_blobs(W, b, question_sequence[i], question_mask[i])
        in_maps.append({"xcT": xcT, "b16": b16, "b32": b32})
    return in_maps


def kernel(context_sequence, question_sequence, question_mask, W, b):
    from concourse.bass_utils import run_bass_kernel_spmd

    nc = _get_nc()
    in_maps = make_in_maps(
        context_sequence, question_sequence, question_mask, W, b)
    res = run_bass_kernel_spmd(nc, in_maps, core_ids=list(range(N_CORES)))
    out = np.stack([res.results[i]["out"] for i in range(N_CORES)], axis=0)
    return out.astype(np.float32)


# revision 11
# speedup vs baseline: 2.6197x; 1.0948x over previous
"""Trainium2 Bass kernel for AlignedQuestionEmbeddingLayer.

Computation (per batch element):
    C = relu(Xc @ W.T + b)            # [4096, 128]
    Q = relu(Xq @ W.T + b)            # [512, 128]
    S = C @ Q.T  (+ mask)             # [4096, 512]
    A = softmax(S, axis=-1)
    out = A @ Q                       # [4096, 128]

Sharding: data-parallel over batch, one batch element per NeuronCore (8 cores).

Device-side design notes:
  - The dense layer contracts over E=300 (padded to 384 on host), so both
    matmul operands need E on the partition axis. Inputs are pre-transposed
    and fp16-cast on host: no on-device transposes of the big tensor, half
    the input DMA traffic, and fp16 matmuls run 4x faster than fp32 on the
    TRN2 PE (1 cycle/row vs 4).
  - One input DMA and one output DMA per 512-row super-tile (the Sync
    sequencer's serial DMA-trigger cost dominates otherwise).
  - Scores are computed transposed ([q partitions, c free]) so the final
    matmul (lhsT=expS_T chunk, rhs=[Q|ones]) produces both the output rows
    and the softmax denominators in a single PSUM accumulation; exp runs as
    two 1024-wide activations (PSUM-bank-spanning reads) with bias=0 when
    the mask is all ones, falling back to 4 per-chunk activations with the
    mask folded into the per-partition bias otherwise.
  - Softmax skips max-subtraction: scores are bounded (~|s|<40) so exp() is
    safe in fp32/bf16.
  - relu(+bias) runs on DVE (tensor_scalar add+max) keeping ACT for exp.
"""

import sys

import numpy as np

sys.path.insert(0, "/opt/trn_rl_repo")

B, CTX, QST, E, H = 8, 4096, 512, 300, 128
N_CORES = 8
EP = 384            # E padded to 3 chunks of 128
ST = 512            # context rows per super-tile
N_ST = CTX // ST    # 8 super-tiles

# fp16 blob free-dim layout: wT chunks, qsT chunks, identity
OFF_W = 0                 # [:, OFF_W + k*128 : +128], k=0..2
OFF_Q = EP                # [:, OFF_Q + k*512 : +512], k=0..2
OFF_ID = OFF_Q + 3 * 512  # identity 128x128
BLOB16_F = OFF_ID + 128
# fp32 blob: bias column + 4 maskbias columns
OFF_B = 0
OFF_MB = 1
BLOB32_F = 5

_COMPILED = {}


def _build_kernel(n_st=N_ST, masked=False):
    import concourse.bass as bass
    import concourse.tile as tile
    from concourse import bacc, mybir

    f32 = mybir.dt.float32
    f16 = mybir.dt.float16
    bf16 = mybir.dt.bfloat16
    AF = mybir.ActivationFunctionType
    ADD = mybir.AluOpType.add
    MAX = mybir.AluOpType.max

    nc = bacc.Bacc(
        "TRN2", target_bir_lowering=False, debug=False, num_devices=N_CORES
    )

    xcT = nc.declare_dram_parameter("xcT", [EP, CTX], f16, isOutput=False)
    b16_d = nc.declare_dram_parameter("b16", [128, BLOB16_F], f16, isOutput=False)
    b32_d = nc.declare_dram_parameter("b32", [128, BLOB32_F], f32, isOutput=False)
    out_d = nc.declare_dram_parameter("out", [CTX, H], f32, isOutput=True)

    with tile.TileContext(nc) as tc:
        with (
            tc.tile_pool(name="const", bufs=1) as const_pool,
            tc.tile_pool(name="xin", bufs=3) as xin_pool,
            tc.tile_pool(name="ct", bufs=2) as ct_pool,
            tc.tile_pool(name="exps", bufs=2) as exps_pool,
            tc.tile_pool(name="outs", bufs=3) as outs_pool,
            tc.tile_pool(name="pct", bufs=1, space=bass.MemorySpace.PSUM) as pct_pool,
            tc.tile_pool(name="pst", bufs=2, space=bass.MemorySpace.PSUM) as pst_pool,
            tc.tile_pool(name="po", bufs=3, space=bass.MemorySpace.PSUM) as po_pool,
        ):
            # ---- one-time setup -------------------------------------------
            setup16 = const_pool.tile([128, BLOB16_F], f16, tag="setup16")
            nc.sync.dma_start(setup16[:], b16_d[:])
            setup32 = const_pool.tile([128, BLOB32_F], f32, tag="setup32")
            nc.sync.dma_start(setup32[:], b32_d[:])

            def w_chunk(k):  # [128, 128] fp16 lhsT for dense
                return setup16[:, OFF_W + k * 128 : OFF_W + (k + 1) * 128]

            def q_chunk(k):  # [128, 512] fp16 rhs for question dense
                return setup16[:, OFF_Q + k * 512 : OFF_Q + (k + 1) * 512]

            ident = setup16[:, OFF_ID : OFF_ID + 128]
            bias_col = setup32[:, OFF_B : OFF_B + 1]

            # question dense: QT = relu(W @ XqT + b) -> [H, QST] fp16
            psum_q = pct_pool.tile([H, QST], f32, tag="pct")
            for k in range(3):
                nc.tensor.matmul(
                    psum_q[:], w_chunk(k), q_chunk(k), start=(k == 0), stop=(k == 2)
                )
            qt_sb = const_pool.tile([H, QST], f16, tag="qt")
            nc.vector.tensor_scalar(qt_sb[:], psum_q[:], bias_col, 0.0, ADD, MAX)

            # Qa[j] = [Q_chunk | ones] -> [128 q, H+1] bf16 (PE transpose of QT)
            qa_sb = []
            for j in range(4):
                pt = po_pool.tile([128, H + 1], f16, tag="po")
                nc.tensor.transpose(
                    pt[:, 0:128], qt_sb[:, j * 128 : (j + 1) * 128], ident
                )
                qa = const_pool.tile([128, H + 1], bf16, tag=f"qa{j}")
                nc.scalar.activation(qa[:, 0:H], pt[:, 0:128], AF.Copy)
                # ones column, written on ACT: Copy(in*0 + 1) = 1
                nc.scalar.activation(
                    qa[:, H : H + 1], bias_col, AF.Copy, bias=1.0, scale=0.0
                )
                qa_sb.append(qa)

            # ---- main loop over context super-tiles -----------------------
            for st in range(n_st):
                c0 = st * ST
                # single Xc load: [EP, ST] as [128, 3, ST]
                xa = xin_pool.tile([128, 3, ST], f16, tag="xa")
                src = xcT[:, c0 : c0 + ST].rearrange("(k p) c -> p k c", p=128)
                nc.sync.dma_start(xa[:], src)

                # context dense: CT = relu(W @ XcT + b) -> [H, ST] fp16 (DVE)
                psum_ct = pct_pool.tile([H, ST], f32, tag="pct")
                for k in range(3):
                    nc.tensor.matmul(
                        psum_ct[:], w_chunk(k), xa[:, k, :],
                        start=(k == 0), stop=(k == 2),
                    )
                ct_sb = ct_pool.tile([H, ST], f16, tag="ct")
                nc.vector.tensor_scalar(ct_sb[:], psum_ct[:], bias_col, 0.0, ADD, MAX)

                # transposed scores (2 PSUM half-tiles) + exp -> bf16
                exp_halves = []
                for half in range(2):
                    ps = pst_pool.tile([128, 2 * ST], f32, tag="pst")
                    for jj in range(2):
                        j = 2 * half + jj
                        nc.tensor.matmul(
                            ps[:, jj * ST : (jj + 1) * ST],
                            qt_sb[:, j * 128 : (j + 1) * 128], ct_sb[:],
                            start=True, stop=True,
                        )
                    es = exps_pool.tile([128, 2 * ST], bf16, tag=f"e{half}")
                    if masked:
                        for jj in range(2):
                            j = 2 * half + jj
                            nc.scalar.activation(
                                es[:, jj * ST : (jj + 1) * ST],
                                ps[:, jj * ST : (jj + 1) * ST], AF.Exp,
                                bias=setup32[:, OFF_MB + j : OFF_MB + j + 1],
                            )
                    else:
                        nc.scalar.activation(es[:], ps[:], AF.Exp)
                    exp_halves.append(es)

                def exp_chunk(ci, j):  # [128 q, 128 c] bf16 lhsT
                    es = exp_halves[j // 2]
                    base = (j % 2) * ST + ci * 128
                    return es[:, base : base + 128]

                # final: out[c,:H] and softmax denominator in one accumulation
                o_big = outs_pool.tile([128, ST // 128, H], f32, tag="obig")
                for ci in range(ST // 128):
                    po = po_pool.tile([128, H + 1], f32, tag="po")
                    for j in range(4):
                        nc.tensor.matmul(
                            po[:], exp_chunk(ci, j), qa_sb[j][:],
                            start=(j == 0), stop=(j == 3),
                        )
                    recip = outs_pool.tile([128, 1], f32, tag="recip")
                    nc.vector.reciprocal(recip[:], po[:, H : H + 1])
                    nc.vector.tensor_scalar_mul(
                        o_big[:, ci, :], po[:, 0:H], recip[:])
                # single store: [128, 4, H] -> out rows c0..c0+512
                dst = out_d[c0 : c0 + ST, :].rearrange("(k p) h -> p k h", p=128)
                nc.sync.dma_start(dst, o_big[:])

    return nc


def _get_nc(masked=False):
    key = ("nc", masked)
    if key not in _COMPILED:
        nc = _build_kernel(masked=masked)
        nc.compile()
        nc.finalize()
        _COMPILED[key] = nc
    return _COMPILED[key]


def make_blobs(W, b, question_sequence_i, question_mask_i):
    """Pack per-core constants into the fp16 and fp32 setup blobs."""
    b16 = np.zeros((128, BLOB16_F), np.float16)
    wTp = np.zeros((EP, H), np.float16)
    wTp[:E] = W.astype(np.float16).T
    for k in range(3):
        b16[:, OFF_W + k * 128 : OFF_W + (k + 1) * 128] = wTp[k * 128 : (k + 1) * 128]
    qTp = np.zeros((EP, QST), np.float16)
    qTp[:E] = question_sequence_i.astype(np.float16).T
    for k in range(3):
        b16[:, OFF_Q + k * 512 : OFF_Q + (k + 1) * 512] = qTp[k * 128 : (k + 1) * 128]
    b16[:, OFF_ID : OFF_ID + 128] = np.eye(128, dtype=np.float16)

    b32 = np.zeros((128, BLOB32_F), np.float32)
    b32[:, OFF_B] = b.astype(np.float32)
    mb = np.where(question_mask_i == 0, np.float32(-1e30), np.float32(0.0))
    b32[:, OFF_MB : OFF_MB + 4] = mb.reshape(4, 128).T
    return b16, b32


def make_in_maps(context_sequence, question_sequence, question_mask, W, b):
    in_maps = []
    for i in range(N_CORES):
        xcT = np.zeros((EP, CTX), np.float16)
        xcT[:E] = context_sequence[i].T.astype(np.float16)
        b16, b32 = make_blobs(W, b, question_sequence[i], question_mask[i])
        in_maps.append({"xcT": xcT, "b16": b16, "b32": b32})
    return in_maps


def kernel(context_sequence, question_sequence, question_mask, W, b):
    from concourse.bass_utils import run_bass_kernel_spmd

    masked = bool(np.any(np.asarray(question_mask) == 0))
    nc = _get_nc(masked=masked)
    in_maps = make_in_maps(
        context_sequence, question_sequence, question_mask, W, b)
    res = run_bass_kernel_spmd(nc, in_maps, core_ids=list(range(N_CORES)))
    out = np.stack([res.results[i]["out"] for i in range(N_CORES)], axis=0)
    return out.astype(np.float32)


# revision 12
# speedup vs baseline: 2.6570x; 1.0142x over previous
"""Trainium2 Bass kernel for AlignedQuestionEmbeddingLayer.

Computation (per batch element):
    C = relu(Xc @ W.T + b)            # [4096, 128]
    Q = relu(Xq @ W.T + b)            # [512, 128]
    S = C @ Q.T  (+ mask)             # [4096, 512]
    A = softmax(S, axis=-1)
    out = A @ Q                       # [4096, 128]

Sharding: data-parallel over batch, one batch element per NeuronCore (8 cores).

Device-side design notes:
  - The dense layer contracts over E=300 (padded to 384 on host), so both
    matmul operands need E on the partition axis. Inputs are pre-transposed
    and fp16-cast on host: no on-device transposes of the big tensor, half
    the input DMA traffic, and fp16 matmuls run 4x faster than fp32 on the
    TRN2 PE (1 cycle/row vs 4).
  - One input DMA and one output DMA per 512-row super-tile (the Sync
    sequencer's serial DMA-trigger cost dominates otherwise).
  - Scores are computed transposed ([q partitions, c free]) so the final
    matmul (lhsT=expS_T chunk, rhs=[Q|ones]) produces both the output rows
    and the softmax denominators in a single PSUM accumulation; exp runs as
    two 1024-wide activations (PSUM-bank-spanning reads) with bias=0 when
    the mask is all ones, falling back to 4 per-chunk activations with the
    mask folded into the per-partition bias otherwise.
  - Softmax skips max-subtraction: scores are bounded (~|s|<40) so exp() is
    safe in fp32/bf16.
  - relu(+bias) runs on DVE (tensor_scalar add+max) keeping ACT for exp.
"""

import sys

import numpy as np

sys.path.insert(0, "/opt/trn_rl_repo")

B, CTX, QST, E, H = 8, 4096, 512, 300, 128
N_CORES = 8
EP = 384            # E padded to 3 chunks of 128
ST = 512            # context rows per super-tile
N_ST = CTX // ST    # 8 super-tiles

# fp16 blob free-dim layout: wT chunks, qsT chunks, identity
OFF_W = 0                 # [:, OFF_W + k*128 : +128], k=0..2
OFF_Q = EP                # [:, OFF_Q + k*512 : +512], k=0..2
OFF_ID = OFF_Q + 3 * 512  # identity 128x128
BLOB16_F = OFF_ID + 128
# fp32 blob: bias column + 4 maskbias columns
OFF_B = 0
OFF_MB = 1
BLOB32_F = 5

_COMPILED = {}


def _build_kernel(n_st=N_ST, masked=False):
    import concourse.bass as bass
    import concourse.tile as tile
    from concourse import bacc, mybir

    f32 = mybir.dt.float32
    f16 = mybir.dt.float16
    bf16 = mybir.dt.bfloat16
    AF = mybir.ActivationFunctionType
    ADD = mybir.AluOpType.add
    MAX = mybir.AluOpType.max

    nc = bacc.Bacc(
        "TRN2", target_bir_lowering=False, debug=False, num_devices=N_CORES
    )

    xcT = nc.declare_dram_parameter("xcT", [EP, CTX], f16, isOutput=False)
    b16_d = nc.declare_dram_parameter("b16", [128, BLOB16_F], f16, isOutput=False)
    b32_d = nc.declare_dram_parameter("b32", [128, BLOB32_F], f32, isOutput=False)
    out_d = nc.declare_dram_parameter("out", [CTX, H], f32, isOutput=True)

    with tile.TileContext(nc) as tc:
        with (
            tc.tile_pool(name="const", bufs=1) as const_pool,
            tc.tile_pool(name="xin", bufs=3) as xin_pool,
            tc.tile_pool(name="ct", bufs=2) as ct_pool,
            tc.tile_pool(name="exps", bufs=2) as exps_pool,
            tc.tile_pool(name="outs", bufs=3) as outs_pool,
            tc.tile_pool(name="pct", bufs=1, space=bass.MemorySpace.PSUM) as pct_pool,
            tc.tile_pool(name="pst", bufs=2, space=bass.MemorySpace.PSUM) as pst_pool,
            tc.tile_pool(name="po", bufs=3, space=bass.MemorySpace.PSUM) as po_pool,
        ):
            # ---- one-time setup -------------------------------------------
            setup16 = const_pool.tile([128, BLOB16_F], f16, tag="setup16")
            nc.sync.dma_start(setup16[:], b16_d[:])
            setup32 = const_pool.tile([128, BLOB32_F], f32, tag="setup32")
            nc.sync.dma_start(setup32[:], b32_d[:])

            def w_chunk(k):  # [128, 128] fp16 lhsT for dense
                return setup16[:, OFF_W + k * 128 : OFF_W + (k + 1) * 128]

            def q_chunk(k):  # [128, 512] fp16 rhs for question dense
                return setup16[:, OFF_Q + k * 512 : OFF_Q + (k + 1) * 512]

            ident = setup16[:, OFF_ID : OFF_ID + 128]
            bias_col = setup32[:, OFF_B : OFF_B + 1]

            # question dense: QT = relu(W @ XqT + b) -> [H, QST] fp16
            psum_q = pct_pool.tile([H, QST], f32, tag="pct")
            for k in range(3):
                nc.tensor.matmul(
                    psum_q[:], w_chunk(k), q_chunk(k), start=(k == 0), stop=(k == 2)
                )
            qt_sb = const_pool.tile([H, QST], f16, tag="qt")
            nc.vector.tensor_scalar(qt_sb[:], psum_q[:], bias_col, 0.0, ADD, MAX)

            # Qa[j] = [Q_chunk | ones] -> [128 q, H+1] bf16 (PE transpose of QT)
            qa_sb = []
            for j in range(4):
                pt = po_pool.tile([128, H + 1], f16, tag="po")
                nc.tensor.transpose(
                    pt[:, 0:128], qt_sb[:, j * 128 : (j + 1) * 128], ident
                )
                qa = const_pool.tile([128, H + 1], bf16, tag=f"qa{j}")
                nc.scalar.activation(qa[:, 0:H], pt[:, 0:128], AF.Copy)
                # ones column, written on ACT: Copy(in*0 + 1) = 1
                nc.scalar.activation(
                    qa[:, H : H + 1], bias_col, AF.Copy, bias=1.0, scale=0.0
                )
                qa_sb.append(qa)

            # ---- main loop over context super-tiles -----------------------
            # Software-pipelined with lag 1: iteration st emits
            #   load/dense/relu(st) -> final/norm/store(st-1) -> scores/exp(st)
            # so the PE never waits on exp (it consumes last iteration's),
            # and relu(st) on DVE hides under the final matmuls of st-1.
            def front_phase(st):
                c0 = st * ST
                xa = xin_pool.tile([128, 3, ST], f16, tag="xa")
                src = xcT[:, c0 : c0 + ST].rearrange("(k p) c -> p k c", p=128)
                nc.sync.dma_start(xa[:], src)

                psum_ct = pct_pool.tile([H, ST], f32, tag="pct")
                for k in range(3):
                    nc.tensor.matmul(
                        psum_ct[:], w_chunk(k), xa[:, k, :],
                        start=(k == 0), stop=(k == 2),
                    )
                ct_sb = ct_pool.tile([H, ST], f16, tag="ct")
                nc.vector.tensor_scalar(ct_sb[:], psum_ct[:], bias_col, 0.0, ADD, MAX)
                return ct_sb

            def scores_exp_phase(ct_sb):
                exp_halves = []
                for half in range(2):
                    ps = pst_pool.tile([128, 2 * ST], f32, tag="pst")
                    for jj in range(2):
                        j = 2 * half + jj
                        nc.tensor.matmul(
                            ps[:, jj * ST : (jj + 1) * ST],
                            qt_sb[:, j * 128 : (j + 1) * 128], ct_sb[:],
                            start=True, stop=True,
                        )
                    es = exps_pool.tile([128, 2 * ST], bf16, tag=f"e{half}")
                    if masked:
                        for jj in range(2):
                            j = 2 * half + jj
                            nc.scalar.activation(
                                es[:, jj * ST : (jj + 1) * ST],
                                ps[:, jj * ST : (jj + 1) * ST], AF.Exp,
                                bias=setup32[:, OFF_MB + j : OFF_MB + j + 1],
                            )
                    else:
                        nc.scalar.activation(es[:], ps[:], AF.Exp)
                    exp_halves.append(es)
                return exp_halves

            def back_phase(st, exp_halves):
                c0 = st * ST

                def exp_chunk(ci, j):  # [128 q, 128 c] bf16 lhsT
                    es = exp_halves[j // 2]
                    base = (j % 2) * ST + ci * 128
                    return es[:, base : base + 128]

                o_big = outs_pool.tile([128, ST // 128, H], f32, tag="obig")
                for ci in range(ST // 128):
                    po = po_pool.tile([128, H + 1], f32, tag="po")
                    for j in range(4):
                        nc.tensor.matmul(
                            po[:], exp_chunk(ci, j), qa_sb[j][:],
                            start=(j == 0), stop=(j == 3),
                        )
                    recip = outs_pool.tile([128, 1], f32, tag="recip")
                    nc.vector.reciprocal(recip[:], po[:, H : H + 1])
                    nc.vector.tensor_scalar_mul(
                        o_big[:, ci, :], po[:, 0:H], recip[:])
                dst = out_d[c0 : c0 + ST, :].rearrange("(k p) h -> p k h", p=128)
                nc.sync.dma_start(dst, o_big[:])

            prev_exp = None
            for st in range(n_st + 1):
                ct_sb = front_phase(st) if st < n_st else None
                if prev_exp is not None:
                    back_phase(st - 1, prev_exp)
                prev_exp = scores_exp_phase(ct_sb) if st < n_st else None

    return nc


def _get_nc(masked=False):
    key = ("nc", masked)
    if key not in _COMPILED:
        nc = _build_kernel(masked=masked)
        nc.compile()
        nc.finalize()
        _COMPILED[key] = nc
    return _COMPILED[key]


def make_blobs(W, b, question_sequence_i, question_mask_i):
    """Pack per-core constants into the fp16 and fp32 setup blobs."""
    b16 = np.zeros((128, BLOB16_F), np.float16)
    wTp = np.zeros((EP, H), np.float16)
    wTp[:E] = W.astype(np.float16).T
    for k in range(3):
        b16[:, OFF_W + k * 128 : OFF_W + (k + 1) * 128] = wTp[k * 128 : (k + 1) * 128]
    qTp = np.zeros((EP, QST), np.float16)
    qTp[:E] = question_sequence_i.astype(np.float16).T
    for k in range(3):
        b16[:, OFF_Q + k * 512 : OFF_Q + (k + 1) * 512] = qTp[k * 128 : (k + 1) * 128]
    b16[:, OFF_ID : OFF_ID + 128] = np.eye(128, dtype=np.float16)

    b32 = np.zeros((128, BLOB32_F), np.float32)
    b32[:, OFF_B] = b.astype(np.float32)
    mb = np.where(question_mask_i == 0, np.float32(-1e30), np.float32(0.0))
    b32[:, OFF_MB : OFF_MB + 4] = mb.reshape(4, 128).T
    return b16, b32


def make_in_maps(context_sequence, question_sequence, question_mask, W, b):
    in_maps = []
    for i in range(N_CORES):
        xcT = np.zeros((EP, CTX), np.float16)
        xcT[:E] = context_sequence[i].T.astype(np.float16)
        b16, b32 = make_blobs(W, b, question_sequence[i], question_mask[i])
        in_maps.append({"xcT": xcT, "b16": b16, "b32": b32})
    return in_maps


def kernel(context_sequence, question_sequence, question_mask, W, b):
    from concourse.bass_utils import run_bass_kernel_spmd

    masked = bool(np.any(np.asarray(question_mask) == 0))
    nc = _get_nc(masked=masked)
    in_maps = make_in_maps(
        context_sequence, question_sequence, question_mask, W, b)
    res = run_bass_kernel_spmd(nc, in_maps, core_ids=list(range(N_CORES)))
    out = np.stack([res.results[i]["out"] for i in range(N_CORES)], axis=0)
    return out.astype(np.float32)


# revision 14
# speedup vs baseline: 2.7209x; 1.0240x over previous
"""Trainium2 Bass kernel for AlignedQuestionEmbeddingLayer.

Computation (per batch element):
    C = relu(Xc @ W.T + b)            # [4096, 128]
    Q = relu(Xq @ W.T + b)            # [512, 128]
    S = C @ Q.T  (+ mask)             # [4096, 512]
    A = softmax(S, axis=-1)
    out = A @ Q                       # [4096, 128]

Sharding: data-parallel over batch, one batch element per NeuronCore (8 cores).

Device-side design notes:
  - The dense layer contracts over E=300 (padded to 384 on host), so both
    matmul operands need E on the partition axis. Inputs are pre-transposed
    and fp16-cast on host: no on-device transposes of the big tensor, half
    the input DMA traffic, and fp16 matmuls run 4x faster than fp32 on the
    TRN2 PE (1 cycle/row vs 4).
  - The bias rides the E-padding: row 300 of xT is all-ones and row 300 of
    the W blob holds b, so both denses produce x@W.T+b straight out of the
    matmul and relu is a single DVE max (fp16/bf16 cast included).
  - One input DMA and one output DMA per 512-row super-tile (the Sync
    sequencer's serial DMA-trigger cost dominates otherwise).
  - Scores are computed transposed ([q partitions, c free]) so the final
    matmul (lhsT=expS_T chunk, rhs=[Q|ones]) produces both the output rows
    and the softmax denominators in a single PSUM accumulation; exp runs as
    two 1024-wide activations with bias=0 when the mask is all ones, and
    falls back to 4 per-chunk activations with the mask folded into the
    per-partition exp bias otherwise.
  - Softmax skips max-subtraction: scores are bounded (~|s|<40) so exp() is
    safe in fp32/bf16.
  - The loop is software-pipelined with lag 1 (final/norm/store of st-1
    between dense and scores of st) so the PE never waits on exp.
  - A burst of dummy matmuls at kernel start warms the PE HAM clock gate
    during the setup-DMA window, so real matmuls run at 2.4 GHz not 1.2.
"""

import sys

import numpy as np

sys.path.insert(0, "/opt/trn_rl_repo")

B, CTX, QST, E, H = 8, 4096, 512, 300, 128
N_CORES = 8
EP = 384            # E padded to 3 chunks of 128; row E carries the bias
ST = 512            # context rows per super-tile
N_ST = CTX // ST    # 8 super-tiles
N_WARM = 18         # dummy matmuls to warm the HAM clock gate

# fp16 blob free-dim layout: wT chunks then qsT chunks
OFF_W = 0                 # [:, OFF_W + k*128 : +128], k=0..2
OFF_Q = EP                # [:, OFF_Q + k*512 : +512], k=0..2
BLOB16_F = OFF_Q + 3 * 512
# fp32 blob: bias column + 4 maskbias columns (only read by masked variant)
OFF_B = 0
OFF_MB = 1
BLOB32_F = 5

_COMPILED = {}


def _build_kernel(n_st=N_ST, masked=False):
    import concourse.bass as bass
    import concourse.tile as tile
    from concourse import bacc, mybir

    f32 = mybir.dt.float32
    f16 = mybir.dt.float16
    bf16 = mybir.dt.bfloat16
    AF = mybir.ActivationFunctionType
    MAX = mybir.AluOpType.max

    nc = bacc.Bacc(
        "TRN2", target_bir_lowering=False, debug=False, num_devices=N_CORES
    )

    xcT = nc.declare_dram_parameter("xcT", [EP, CTX], f16, isOutput=False)
    b16_d = nc.declare_dram_parameter("b16", [128, BLOB16_F], f16, isOutput=False)
    b32_d = nc.declare_dram_parameter("b32", [128, BLOB32_F], f32, isOutput=False)
    out_d = nc.declare_dram_parameter("out", [CTX, H], f32, isOutput=True)

    with tile.TileContext(nc) as tc:
        with (
            tc.tile_pool(name="const", bufs=1) as const_pool,
            tc.tile_pool(name="xin", bufs=4) as xin_pool,
            tc.tile_pool(name="ct", bufs=2) as ct_pool,
            tc.tile_pool(name="exps", bufs=2) as exps_pool,
            tc.tile_pool(name="outs", bufs=3) as outs_pool,
            tc.tile_pool(name="pct", bufs=1, space=bass.MemorySpace.PSUM) as pct_pool,
            tc.tile_pool(name="pst", bufs=2, space=bass.MemorySpace.PSUM) as pst_pool,
            tc.tile_pool(name="po", bufs=3, space=bass.MemorySpace.PSUM) as po_pool,
        ):
            # ---- PE warmup: matmuls on an uninitialized tile, results
            # discarded (next dense's start=True clears the bank) ----------
            warm = const_pool.tile([128, ST], f16, tag="warm")
            nc.gpsimd.memset(warm[:], 0.0)
            warm_ps = pct_pool.tile([H, ST], f32, tag="pct")
            for _ in range(N_WARM):
                nc.tensor.matmul(
                    warm_ps[:], warm[:, 0:128], warm[:], start=True, stop=True,
                    skip_group_check=True,
                )

            # ---- one-time setup -------------------------------------------
            setup16 = const_pool.tile([128, BLOB16_F], f16, tag="setup16")
            nc.sync.dma_start(setup16[:], b16_d[:])
            if masked:
                setup32 = const_pool.tile([128, BLOB32_F], f32, tag="setup32")
                nc.sync.dma_start(setup32[:], b32_d[:])

            def w_chunk(k):  # [128, 128] fp16 W.T chunk (row E holds b)
                return setup16[:, OFF_W + k * 128 : OFF_W + (k + 1) * 128]

            def q_chunk(k):  # [128, 512] fp16 Xq.T chunk (row E all-ones)
                return setup16[:, OFF_Q + k * 512 : OFF_Q + (k + 1) * 512]

            # question dense twice (bias rides the aug row):
            #   QT [h, q] fp16 for the scores lhsT
            #   Q  [q, h] bf16 (as [Q|ones] per chunk) for the final rhs
            psum_q = pct_pool.tile([H, QST], f32, tag="pct")
            for k in range(3):
                nc.tensor.matmul(
                    psum_q[:], w_chunk(k), q_chunk(k), start=(k == 0), stop=(k == 2)
                )
            qt_sb = const_pool.tile([H, QST], f16, tag="qt")
            nc.vector.tensor_scalar(qt_sb[:], psum_q[:], 0.0, None, MAX)

            psum_qd = pst_pool.tile([128, 2 * ST], f32, tag="pst")
            for j in range(4):
                for k in range(3):
                    nc.tensor.matmul(
                        psum_qd[:, j * 128 : (j + 1) * 128],
                        q_chunk(k)[:, j * 128 : (j + 1) * 128], w_chunk(k),
                        start=(k == 0), stop=(k == 2),
                    )
            qa_sb = []
            for j in range(4):
                qa = const_pool.tile([128, H + 1], bf16, tag=f"qa{j}")
                nc.vector.tensor_scalar(
                    qa[:, 0:H], psum_qd[:, j * 128 : (j + 1) * 128], 0.0, None, MAX
                )
                # ones column, written on ACT: Copy(in*0 + 1) = 1
                nc.scalar.activation(
                    qa[:, H : H + 1], setup16[:, 0:1], AF.Copy, bias=1.0, scale=0.0
                )
                qa_sb.append(qa)

            # ---- software-pipelined main loop -----------------------------
            def front_phase(st):
                c0 = st * ST
                xa = xin_pool.tile([128, 3, ST], f16, tag="xa")
                src = xcT[:, c0 : c0 + ST].rearrange("(k p) c -> p k c", p=128)
                nc.sync.dma_start(xa[:], src)

                psum_ct = pct_pool.tile([H, ST], f32, tag="pct")
                for k in range(3):
                    nc.tensor.matmul(
                        psum_ct[:], w_chunk(k), xa[:, k, :],
                        start=(k == 0), stop=(k == 2),
                    )
                ct_sb = ct_pool.tile([H, ST], f16, tag="ct")
                nc.vector.tensor_scalar(ct_sb[:], psum_ct[:], 0.0, None, MAX)
                return ct_sb

            def scores_exp_phase(ct_sb):
                exp_halves = []
                for half in range(2):
                    ps = pst_pool.tile([128, 2 * ST], f32, tag="pst")
                    for jj in range(2):
                        j = 2 * half + jj
                        nc.tensor.matmul(
                            ps[:, jj * ST : (jj + 1) * ST],
                            qt_sb[:, j * 128 : (j + 1) * 128], ct_sb[:],
                            start=True, stop=True,
                        )
                    es = exps_pool.tile([128, 2 * ST], bf16, tag=f"e{half}")
                    if masked:
                        for jj in range(2):
                            j = 2 * half + jj
                            nc.scalar.activation(
                                es[:, jj * ST : (jj + 1) * ST],
                                ps[:, jj * ST : (jj + 1) * ST], AF.Exp,
                                bias=setup32[:, OFF_MB + j : OFF_MB + j + 1],
                            )
                    else:
                        nc.scalar.activation(es[:], ps[:], AF.Exp)
                    exp_halves.append(es)
                return exp_halves

            def back_phase(st, exp_halves):
                c0 = st * ST

                def exp_chunk(ci, j):  # [128 q, 128 c] bf16 lhsT
                    es = exp_halves[j // 2]
                    base = (j % 2) * ST + ci * 128
                    return es[:, base : base + 128]

                o_big = outs_pool.tile([128, ST // 128, H], f32, tag="obig")
                for ci in range(ST // 128):
                    po = po_pool.tile([128, H + 1], f32, tag="po")
                    for j in range(4):
                        nc.tensor.matmul(
                            po[:], exp_chunk(ci, j), qa_sb[j][:],
                            start=(j == 0), stop=(j == 3),
                        )
                    recip = outs_pool.tile([128, 1], f32, tag="recip")
                    nc.vector.reciprocal(recip[:], po[:, H : H + 1])
                    nc.vector.tensor_scalar_mul(
                        o_big[:, ci, :], po[:, 0:H], recip[:])
                dst = out_d[c0 : c0 + ST, :].rearrange("(k p) h -> p k h", p=128)
                nc.sync.dma_start(dst, o_big[:])

            prev_exp = None
            for st in range(n_st + 1):
                ct_sb = front_phase(st) if st < n_st else None
                if prev_exp is not None:
                    back_phase(st - 1, prev_exp)
                prev_exp = scores_exp_phase(ct_sb) if st < n_st else None

    return nc


def _get_nc(masked=False):
    key = ("nc", masked)
    if key not in _COMPILED:
        nc = _build_kernel(masked=masked)
        nc.compile()
        nc.finalize()
        _COMPILED[key] = nc
    return _COMPILED[key]


def make_blobs(W, b, question_sequence_i, question_mask_i):
    """Pack per-core constants into the fp16 and fp32 setup blobs."""
    b16 = np.zeros((128, BLOB16_F), np.float16)
    wTp = np.zeros((EP, H), np.float16)
    wTp[:E] = W.astype(np.float16).T
    wTp[E] = b.astype(np.float16)          # bias rides the aug row
    for k in range(3):
        b16[:, OFF_W + k * 128 : OFF_W + (k + 1) * 128] = wTp[k * 128 : (k + 1) * 128]
    qTp = np.zeros((EP, QST), np.float16)
    qTp[:E] = question_sequence_i.astype(np.float16).T
    qTp[E] = 1.0                           # ones row pairs with the bias row
    for k in range(3):
        b16[:, OFF_Q + k * 512 : OFF_Q + (k + 1) * 512] = qTp[k * 128 : (k + 1) * 128]

    b32 = np.zeros((128, BLOB32_F), np.float32)
    b32[:, OFF_B] = b.astype(np.float32)
    mb = np.where(question_mask_i == 0, np.float32(-1e30), np.float32(0.0))
    b32[:, OFF_MB : OFF_MB + 4] = mb.reshape(4, 128).T
    return b16, b32


def make_in_maps(context_sequence, question_sequence, question_mask, W, b):
    in_maps = []
    for i in range(N_CORES):
        xcT = np.zeros((EP, CTX), np.float16)
        xcT[:E] = context_sequence[i].T.astype(np.float16)
        xcT[E] = 1.0                       # ones row pairs with the bias row
        b16, b32 = make_blobs(W, b, question_sequence[i], question_mask[i])
        in_maps.append({"xcT": xcT, "b16": b16, "b32": b32})
    return in_maps


def kernel(context_sequence, question_sequence, question_mask, W, b):
    from concourse.bass_utils import run_bass_kernel_spmd

    masked = bool(np.any(np.asarray(question_mask) == 0))
    nc = _get_nc(masked=masked)
    in_maps = make_in_maps(
        context_sequence, question_sequence, question_mask, W, b)
    res = run_bass_kernel_spmd(nc, in_maps, core_ids=list(range(N_CORES)))
    out = np.stack([res.results[i]["out"] for i in range(N_CORES)], axis=0)
    return out.astype(np.float32)


# revision 17
# speedup vs baseline: 2.8122x; 1.0336x over previous
"""Trainium2 Bass kernel for AlignedQuestionEmbeddingLayer.

Computation (per batch element):
    C = relu(Xc @ W.T + b)            # [4096, 128]
    Q = relu(Xq @ W.T + b)            # [512, 128]
    S = C @ Q.T  (+ mask)             # [4096, 512]
    A = softmax(S, axis=-1)
    out = A @ Q                       # [4096, 128]

Sharding: data-parallel over batch, one batch element per NeuronCore (8 cores).

Device-side design notes:
  - The dense layer contracts over E=300 (padded to 384 on host), so both
    matmul operands need E on the partition axis. Inputs are pre-transposed
    and fp16-cast on host: no on-device transposes of the big tensor, half
    the input DMA traffic, and fp16 matmuls run 4x faster than fp32 on the
    TRN2 PE (1 cycle/row vs 4).
  - The bias rides the E-padding: row 300 of xT is all-ones and row 300 of
    the W blob holds b, so both denses produce x@W.T+b straight out of the
    matmul and relu is a single DVE max (fp16/bf16 cast included).
  - One input DMA and one output DMA per 512-row super-tile (the Sync
    sequencer's serial DMA-trigger cost dominates otherwise).
  - Scores are computed transposed ([q partitions, c free]) so the final
    matmul (lhsT=expS_T chunk, rhs=[Q|ones]) produces both the output rows
    and the softmax denominators in a single PSUM accumulation; exp runs as
    two 1024-wide activations with bias=0 when the mask is all ones, and
    falls back to 4 per-chunk activations with the mask folded into the
    per-partition exp bias otherwise.
  - Softmax skips max-subtraction: scores are bounded (~|s|<40) so exp() is
    safe in fp32/bf16.
  - The loop is software-pipelined with lag 1 (final/norm/store of st-1
    between dense and scores of st) so the PE never waits on exp.
  - A burst of dummy matmuls at kernel start warms the PE HAM clock gate
    during the setup-DMA window, so real matmuls run at 2.4 GHz not 1.2.
"""

import sys

import numpy as np

sys.path.insert(0, "/opt/trn_rl_repo")

B, CTX, QST, E, H = 8, 4096, 512, 300, 128
N_CORES = 8
EP = 384            # E padded to 3 chunks of 128; row E carries the bias
ST = 512            # context rows per super-tile
N_ST = CTX // ST    # 8 super-tiles
N_WARM = 6          # dummy matmuls to warm the HAM clock gate

# fp16 blob free-dim layout: wT chunks then qsT chunks
OFF_W = 0                 # [:, OFF_W + k*128 : +128], k=0..2
OFF_Q = EP                # [:, OFF_Q + k*512 : +512], k=0..2
BLOB16_F = OFF_Q + 3 * 512
# fp32 blob: bias column + 4 maskbias columns (only read by masked variant)
OFF_B = 0
OFF_MB = 1
BLOB32_F = 5

_COMPILED = {}


def _build_kernel(n_st=N_ST, masked=False):
    import concourse.bass as bass
    import concourse.tile as tile
    from concourse import bacc, mybir

    f32 = mybir.dt.float32
    f16 = mybir.dt.float16
    bf16 = mybir.dt.bfloat16
    AF = mybir.ActivationFunctionType
    MAX = mybir.AluOpType.max

    nc = bacc.Bacc(
        "TRN2", target_bir_lowering=False, debug=False, num_devices=N_CORES
    )

    xcT = nc.declare_dram_parameter("xcT", [EP, CTX], f16, isOutput=False)
    b16_d = nc.declare_dram_parameter("b16", [128, BLOB16_F], f16, isOutput=False)
    b32_d = nc.declare_dram_parameter("b32", [128, BLOB32_F], f32, isOutput=False)
    out_d = nc.declare_dram_parameter("out", [CTX, H], f32, isOutput=True)

    with tile.TileContext(nc) as tc:
        with (
            tc.tile_pool(name="const", bufs=1) as const_pool,
            tc.tile_pool(name="xin", bufs=4) as xin_pool,
            tc.tile_pool(name="ct", bufs=2) as ct_pool,
            tc.tile_pool(name="exps", bufs=2) as exps_pool,
            tc.tile_pool(name="outs", bufs=3) as outs_pool,
            tc.tile_pool(name="pct", bufs=1, space=bass.MemorySpace.PSUM) as pct_pool,
            tc.tile_pool(name="pst", bufs=2, space=bass.MemorySpace.PSUM) as pst_pool,
            tc.tile_pool(name="po", bufs=3, space=bass.MemorySpace.PSUM) as po_pool,
        ):
            # ---- PE warmup: matmuls on an uninitialized tile, results
            # discarded (next dense's start=True clears the bank) ----------
            warm = const_pool.tile([128, ST], f16, tag="warm")
            nc.gpsimd.memset(warm[:], 0.0)
            warm_ps = pct_pool.tile([H, ST], f32, tag="pct")
            for _ in range(N_WARM):
                nc.tensor.matmul(
                    warm_ps[:], warm[:, 0:128], warm[:], start=True, stop=True,
                    skip_group_check=True,
                )

            # ---- one-time setup -------------------------------------------
            setup16 = const_pool.tile([128, BLOB16_F], f16, tag="setup16")
            nc.sync.dma_start(setup16[:], b16_d[:])
            if masked:
                setup32 = const_pool.tile([128, BLOB32_F], f32, tag="setup32")
                nc.sync.dma_start(setup32[:], b32_d[:])

            def w_chunk(k):  # [128, 128] fp16 W.T chunk (row E holds b)
                return setup16[:, OFF_W + k * 128 : OFF_W + (k + 1) * 128]

            def q_chunk(k):  # [128, 512] fp16 Xq.T chunk (row E all-ones)
                return setup16[:, OFF_Q + k * 512 : OFF_Q + (k + 1) * 512]

            # question dense twice (bias rides the aug row):
            #   QT [h, q] fp16 for the scores lhsT
            #   Q  [q, h] bf16 (as [Q|ones] per chunk) for the final rhs
            qt_sb = const_pool.tile([H, QST], f16, tag="qt")
            qa_sb = []
            for j in range(4):
                qa_tile = const_pool.tile([128, H + 1], bf16, tag=f"qa{j}")
                qa_sb.append(qa_tile)

            def q_setup():
                psum_q = pct_pool.tile([H, QST], f32, tag="pct")
                for k in range(3):
                    nc.tensor.matmul(
                        psum_q[:], w_chunk(k), q_chunk(k),
                        start=(k == 0), stop=(k == 2),
                    )
                nc.vector.tensor_scalar(qt_sb[:], psum_q[:], 0.0, None, MAX)

                psum_qd = pst_pool.tile([128, 2 * ST], f32, tag="pst")
                for j in range(4):
                    for k in range(3):
                        nc.tensor.matmul(
                            psum_qd[:, j * 128 : (j + 1) * 128],
                            q_chunk(k)[:, j * 128 : (j + 1) * 128], w_chunk(k),
                            start=(k == 0), stop=(k == 2),
                        )
                for j in range(4):
                    nc.vector.tensor_scalar(
                        qa_sb[j][:, 0:H], psum_qd[:, j * 128 : (j + 1) * 128],
                        0.0, None, MAX,
                    )
                    nc.gpsimd.memset(qa_sb[j][:, H : H + 1], 1.0)

            # ---- software-pipelined main loop -----------------------------
            def front_phase(st):
                c0 = st * ST
                xa = xin_pool.tile([128, 3, ST], f16, tag="xa")
                src = xcT[:, c0 : c0 + ST].rearrange("(k p) c -> p k c", p=128)
                nc.sync.dma_start(xa[:], src)

                psum_ct = pct_pool.tile([H, ST], f32, tag="pct")
                for k in range(3):
                    nc.tensor.matmul(
                        psum_ct[:], w_chunk(k), xa[:, k, :],
                        start=(k == 0), stop=(k == 2),
                    )
                ct_sb = ct_pool.tile([H, ST], f16, tag="ct")
                nc.vector.tensor_scalar(ct_sb[:], psum_ct[:], 0.0, None, MAX)
                return ct_sb

            def scores_exp_phase(ct_sb):
                exp_halves = []
                for half in range(2):
                    ps = pst_pool.tile([128, 2 * ST], f32, tag="pst")
                    for jj in range(2):
                        j = 2 * half + jj
                        nc.tensor.matmul(
                            ps[:, jj * ST : (jj + 1) * ST],
                            qt_sb[:, j * 128 : (j + 1) * 128], ct_sb[:],
                            start=True, stop=True,
                        )
                    es = exps_pool.tile([128, 2 * ST], bf16, tag=f"e{half}")
                    if masked:
                        for jj in range(2):
                            j = 2 * half + jj
                            nc.scalar.activation(
                                es[:, jj * ST : (jj + 1) * ST],
                                ps[:, jj * ST : (jj + 1) * ST], AF.Exp,
                                bias=setup32[:, OFF_MB + j : OFF_MB + j + 1],
                            )
                    else:
                        nc.scalar.activation(es[:], ps[:], AF.Exp)
                    exp_halves.append(es)
                return exp_halves

            def back_phase(st, exp_halves):
                c0 = st * ST

                def exp_chunk(ci, j):  # [128 q, 128 c] bf16 lhsT
                    es = exp_halves[j // 2]
                    base = (j % 2) * ST + ci * 128
                    return es[:, base : base + 128]

                o_big = outs_pool.tile([128, ST // 128, H], f32, tag="obig")
                for ci in range(ST // 128):
                    po = po_pool.tile([128, H + 1], f32, tag="po")
                    for j in range(4):
                        nc.tensor.matmul(
                            po[:], exp_chunk(ci, j), qa_sb[j][:],
                            start=(j == 0), stop=(j == 3),
                        )
                    recip = outs_pool.tile([128, 1], f32, tag="recip")
                    nc.vector.reciprocal(recip[:], po[:, H : H + 1])
                    nc.vector.tensor_scalar_mul(
                        o_big[:, ci, :], po[:, 0:H], recip[:])
                dst = out_d[c0 : c0 + ST, :].rearrange("(k p) h -> p k h", p=128)
                nc.sync.dma_start(dst, o_big[:])

            # prologue: two fronts, then the question dense, then pipeline
            # with front two super-tiles ahead so relu/exp always hide
            cts = {0: front_phase(0)}
            if n_st > 1:
                cts[1] = front_phase(1)
            q_setup()
            prev_exp = scores_exp_phase(cts[0])
            for st in range(1, n_st + 1):
                if st < n_st:
                    if st + 1 < n_st:
                        cts[st + 1] = front_phase(st + 1)
                    back_phase(st - 1, prev_exp)
                    prev_exp = scores_exp_phase(cts.pop(st))
                else:
                    back_phase(st - 1, prev_exp)

    return nc


def _get_nc(masked=False):
    key = ("nc", masked)
    if key not in _COMPILED:
        nc = _build_kernel(masked=masked)
        nc.compile()
        nc.finalize()
        _COMPILED[key] = nc
    return _COMPILED[key]


def make_blobs(W, b, question_sequence_i, question_mask_i):
    """Pack per-core constants into the fp16 and fp32 setup blobs."""
    b16 = np.zeros((128, BLOB16_F), np.float16)
    wTp = np.zeros((EP, H), np.float16)
    wTp[:E] = W.astype(np.float16).T
    wTp[E] = b.astype(np.float16)          # bias rides the aug row
    for k in range(3):
        b16[:, OFF_W + k * 128 : OFF_W + (k + 1) * 128] = wTp[k * 128 : (k + 1) * 128]
    qTp = np.zeros((EP, QST), np.float16)
    qTp[:E] = question_sequence_i.astype(np.float16).T
    qTp[E] = 1.0                           # ones row pairs with the bias row
    for k in range(3):
        b16[:, OFF_Q + k * 512 : OFF_Q + (k + 1) * 512] = qTp[k * 128 : (k + 1) * 128]

    b32 = np.zeros((128, BLOB32_F), np.float32)
    b32[:, OFF_B] = b.astype(np.float32)
    mb = np.where(question_mask_i == 0, np.float32(-1e30), np.float32(0.0))
    b32[:, OFF_MB : OFF_MB + 4] = mb.reshape(4, 128).T
    return b16, b32


def make_in_maps(context_sequence, question_sequence, question_mask, W, b):
    in_maps = []
    for i in range(N_CORES):
        xcT = np.zeros((EP, CTX), np.float16)
        xcT[:E] = context_sequence[i].T.astype(np.float16)
        xcT[E] = 1.0                       # ones row pairs with the bias row
        b16, b32 = make_blobs(W, b, question_sequence[i], question_mask[i])
        in_maps.append({"xcT": xcT, "b16": b16, "b32": b32})
    return in_maps


def kernel(context_sequence, question_sequence, question_mask, W, b):
    from concourse.bass_utils import run_bass_kernel_spmd

    masked = bool(np.any(np.asarray(question_mask) == 0))
    nc = _get_nc(masked=masked)
    in_maps = make_in_maps(
        context_sequence, question_sequence, question_mask, W, b)
    res = run_bass_kernel_spmd(nc, in_maps, core_ids=list(range(N_CORES)))
    out = np.stack([res.results[i]["out"] for i in range(N_CORES)], axis=0)
    return out.astype(np.float32)
